# revision 15
# baseline (speedup 1.0000x reference)
"""Trainium2 Bass kernel for the trajectory-decoder LSTM problem.

Math (mirrors the reference, with algebraic folds):
  dec_inp feeds gates only through W_emb; W_sp/W_emb/W_hp collapse:
    W_es = W_emb @ W_sp            [4H, 2]
    gates_t = zx@W_zx.T + bias + r_{t-1}@W_es.T + h_{t-1}@W_hh.T
  For t>=1, r_{t-1} = h_{t-1}@W_hp.T + b_hp, so with
    W_hh' = W_hh + W_es @ W_hp,  bias1 = b_ih + b_hh + W_emb@b_sp + W_es@b_hp
  every step becomes uniform:  gates_t = zx@W_zx.T + bias1 + h_{t-1}@W_hh'.T
  plus a rank-2 step-0 correction (lpr - r_init)@W_es.T injected once.
  `last_pos` is dead code (output is just the stacked rel_pos).

Device strategy (v2: 512-wide pair-waves, g-gate recomputed per step):
  - batch on the free dim, features on partitions; all matmuls N=512.
  - per 512-batch pair-wave, the i/f/o gate pre-activations live RESIDENT
    in a 3-bank PSUM tile; each step the PE accumulates
    (h_t - h_{t-1}) @ W_hh'[ifo].T into it.  The g (cell-candidate) gate
    is instead recomputed fresh each step into a transient 1-bank PSUM
    scratch slot: g_t = zxp_g(SBUF) + h_t @ W_hh'_g.T.  This shrinks the
    resident footprint so TWO pair-chains (2x3 banks) plus a 2-buf
    scratch pool (MLP/psr/g/rel) exactly fill the 8 PSUM banks.
  - bias1 is folded into the zx projection via a constant-1.0 row of the
    padded zx input (row 1056), killing the bank-open bias matmuls.
  - the g-gate weights are doubled on host so sigmoid(2g) gives
    tanh(g) = 2*sig(2g)-1 via the DVE ops; same trick for tanh(c) input.
  - rel_pos matmuls (M=2) are col-packed 4x with tile_position so four
    128x2x512 products run concurrently in different column groups.
  - waves are software-pipelined with a staggered round schedule
    (2 pair-chains in flight); Tile executes each engine's stream in
    emission order.
  - all matmul operands fp16 (full PE rate), PSUM accumulation fp32;
    c kept in fp16, b_hp added on host after gather.
"""

import os
import numpy as np

B = 32768
NCORES = 8
BC = B // NCORES          # 4096 batch per core
W = 512                   # batch per pair-chain
NP = BC // W              # 8 pair-waves per core
T = 12                    # decode steps
H = 128
G4 = 4 * H                # 512 gate features
G3 = 3 * H                # i,f,o resident gates
ZX = 1056
KP = 1152                 # ZX padded to 9*128 (row 1056 = 1.0 carries bias1)
KT = KP // 128            # 9 contraction tiles
MLP = 1024
EMB = 64

_cache = {}


def _build_nc():
    import concourse.bass as bass
    import concourse.bacc as bacc
    import concourse.mybir as mybir
    import concourse.tile as tile
    from concourse.bass import ts

    f16 = mybir.dt.float16
    f32 = mybir.dt.float32
    AF = mybir.ActivationFunctionType
    OP = mybir.AluOpType

    nc = bacc.Bacc("TRN2", target_bir_lowering=False)

    zxT = nc.dram_tensor("zxT", [KP, BC], f16, kind="ExternalInput")
    lprT = nc.dram_tensor("lprT", [2, BC], f16, kind="ExternalInput")
    w1t = nc.dram_tensor("w1t", [128, KT, MLP], f16, kind="ExternalInput")
    wzxt = nc.dram_tensor("wzxt", [128, KT, G4], f16, kind="ExternalInput")
    w2t = nc.dram_tensor("w2t", [128, 8, H], f16, kind="ExternalInput")
    whht = nc.dram_tensor("whht", [128, G4], f16, kind="ExternalInput")
    whpt = nc.dram_tensor("whpt", [128, 2], f16, kind="ExternalInput")
    k3 = nc.dram_tensor("k3", [2, G4], f16, kind="ExternalInput")   # -W_es.T
    wes = nc.dram_tensor("wes", [2, G3], f16, kind="ExternalInput")  # +W_es.T ifo
    b1 = nc.dram_tensor("b1", [128, 8], f32, kind="ExternalInput")
    b2 = nc.dram_tensor("b2", [128, 1], f32, kind="ExternalInput")
    bhp = nc.dram_tensor("bhp", [2, 1], f32, kind="ExternalInput")
    pred = nc.dram_tensor("pred", [T, 2, BC], f32, kind="ExternalOutput")

    with tile.TileContext(nc) as tc:
        with (
            tc.tile_pool(name="consts", bufs=1) as cpool,
            tc.tile_pool(name="zx", bufs=2) as zxpool,
            tc.tile_pool(name="h1", bufs=2) as h1pool,
            tc.tile_pool(name="hc", bufs=5) as hcpool,
            tc.tile_pool(name="acts", bufs=4) as apool,
            tc.tile_pool(name="hall", bufs=2) as hallpool,
            tc.tile_pool(name="preds", bufs=2) as predpool,
            tc.tile_pool(name="scrps", bufs=2, space="PSUM") as scrpool,
            tc.tile_pool(name="gateps", bufs=2, space="PSUM") as gatepool,
        ):
            # ---- load constants once ----
            w1t_s = cpool.tile([128, KT, MLP], f16)
            nc.sync.dma_start(w1t_s[:], w1t[:])
            wzxt_s = cpool.tile([128, KT, G4], f16)
            nc.sync.dma_start(wzxt_s[:], wzxt[:])
            w2t_s = cpool.tile([128, 8, H], f16)
            nc.sync.dma_start(w2t_s[:], w2t[:])
            whht_s = cpool.tile([128, G4], f16)
            nc.sync.dma_start(whht_s[:], whht[:])
            whpt_s = cpool.tile([128, 2], f16)
            nc.sync.dma_start(whpt_s[:], whpt[:])
            k3_s = cpool.tile([2, G4], f16)
            nc.sync.dma_start(k3_s[:], k3[:])
            wes_s = cpool.tile([2, G3], f16)
            nc.sync.dma_start(wes_s[:], wes[:])
            b1_s = cpool.tile([128, 8], f32)
            nc.sync.dma_start(b1_s[:], b1[:])
            b2_s = cpool.tile([128, 1], f32)
            nc.sync.dma_start(b2_s[:], b2[:])
            bhp_s = cpool.tile([2, 1], f32)
            nc.sync.dma_start(bhp_s[:], bhp[:])
            lpr_s = cpool.tile([2, BC], f16)
            nc.sync.dma_start(lpr_s[:], lprT[:])

            zxT_v = zxT.rearrange("(k p) b -> p k b", p=128)
            # pred viewed as [q, j, comp, b] with t = 4*q + j (rel col-packing)
            pred_v = pred.rearrange("(q j) c b -> q j c b", j=4)

            state = [dict() for _ in range(NP)]
            events = []  # (round, prio, fn)

            def mk_mlp1(p, j):
                def fn():
                    st = state[p]
                    if "zxw" not in st:
                        zxw = zxpool.tile([128, KT, W], f16, tag="zxw", name="zxw")
                        nc.sync.dma_start(zxw[:], zxT_v[:, :, ts(p, W)])
                        st["zxw"] = zxw
                        st["h1"] = h1pool.tile([128, 8, W], f16, tag="h1", name="h1")
                    ps = scrpool.tile([128, W], f32, tag="scratch", name="ps")
                    for k in range(KT):
                        nc.tensor.matmul(
                            ps[:], w1t_s[:, k, ts(j, 128)], st["zxw"][:, k, :],
                            start=(k == 0), stop=(k == KT - 1),
                        )
                    nc.vector.tensor_scalar(
                        st["h1"][:, j, :], ps[:], b1_s[:, j : j + 1], 0.0,
                        OP.add, OP.max,
                    )
                return fn

            def mk_mlp2(p):
                def fn():
                    st = state[p]
                    ps = scrpool.tile([128, W], f32, tag="scratch", name="ps")
                    for j in range(8):
                        nc.tensor.matmul(
                            ps[:], w2t_s[:, j, :], st["h1"][:, j, :],
                            start=(j == 0), stop=(j == 7),
                        )
                    hi = h1pool.tile([128, W], f16, tag="hinit", name="hinit")
                    nc.vector.tensor_scalar(
                        hi[:], ps[:], b2_s[:, 0:1], 0.0, OP.add, OP.max
                    )
                    st["h_prev"] = hi
                return fn

            def mk_prep(p):
                # psr -> k3rhs, and zxp_g -> SBUF (g-gate zx projection)
                def fn():
                    st = state[p]
                    ps = scrpool.tile([128, W], f32, tag="scratch", name="ps")
                    nc.tensor.matmul(
                        ps[0:2, :], whpt_s[:], st["h_prev"][:], start=True, stop=True
                    )
                    k3rhs = apool.tile([2, W], f16, tag="k3rhs", name="k3rhs")
                    nc.vector.scalar_tensor_tensor(
                        k3rhs[:], ps[0:2, :], bhp_s[:, 0:1], lpr_s[:, ts(p, W)],
                        OP.add, OP.subtract,
                    )
                    st["k3rhs"] = k3rhs
                    ps2 = scrpool.tile([128, W], f32, tag="scratch", name="ps")
                    for k in range(KT):
                        nc.tensor.matmul(
                            ps2[:], wzxt_s[:, k, G3:G4], st["zxw"][:, k, :],
                            start=(k == 0), stop=(k == KT - 1),
                        )
                    zxpg = h1pool.tile([128, W], f16, tag="zxpg", name="zxpg")
                    nc.vector.tensor_copy(zxpg[:], ps2[:])
                    st["zxpg"] = zxpg
                return fn

            def mk_init1(p):
                def fn():
                    st = state[p]
                    gates = gatepool.tile([128, G3 // 128 * W], f32, tag="gates", name="gates")
                    st["gates"] = gates
                    for g in range(2):
                        gp = gates[:, ts(g, W)]
                        for k in range(KT):
                            nc.tensor.matmul(
                                gp[:], wzxt_s[:, k, ts(g, 128)],
                                st["zxw"][:, k, :],
                                start=(k == 0), stop=False, skip_group_check=True,
                            )
                return fn

            def mk_init2(p):
                def fn():
                    st = state[p]
                    gates = st["gates"]
                    gp = gates[:, ts(2, W)]
                    for k in range(KT):
                        nc.tensor.matmul(
                            gp[:], wzxt_s[:, k, ts(2, 128)], st["zxw"][:, k, :],
                            start=(k == 0), stop=False, skip_group_check=True,
                        )
                    for g in range(3):
                        gp = gates[:, ts(g, W)]
                        nc.tensor.matmul(
                            gp[:], whht_s[:, ts(g, 128)], st["h_prev"][:],
                            start=False, stop=False, skip_group_check=True,
                        )
                        nc.tensor.matmul(
                            gp[:], k3_s[:, ts(g, 128)], st["k3rhs"][:],
                            start=False, stop=False, skip_group_check=True,
                        )
                    st["h_all"] = hallpool.tile([128, T * W], f16, tag="hall", name="hall")
                return fn

            def mk_s1(p, t):
                def fn():
                    st = state[p]
                    # fresh g-gate: psum <- h_prev @ Whh'_g (+ step-0 corr)
                    psg = scrpool.tile([128, W], f32, tag="scratch", name="psg")
                    nc.tensor.matmul(
                        psg[:], whht_s[:, G3:G4], st["h_prev"][:],
                        start=True, stop=(t > 0), skip_group_check=True,
                    )
                    if t == 0:
                        nc.tensor.matmul(
                            psg[:], k3_s[:, G3:G4], st["k3rhs"][:],
                            start=False, stop=True, skip_group_check=True,
                        )
                    # gsum = g pre-activation = psum + zxp_g ; tanh directly
                    gsum = apool.tile([128, W], f16, tag="gsum", name="gsum")
                    nc.vector.tensor_tensor(gsum[:], psg[:], st["zxpg"][:], OP.add)
                    sig = apool.tile([128, 3 * W], f16, tag="sig", name="sig")
                    # sig(i) alone first: it gates the critical path via m1
                    nc.scalar.activation(sig[:, 0:W], st["gates"][:, 0:W], AF.Sigmoid)
                    tg = apool.tile([128, W], f16, tag="tg", name="tg")
                    nc.scalar.activation(tg[:], gsum[:], AF.Tanh)
                    # m1 = sig(i) * tanh(g)   (plain tensor_tensor -> 2x DVE rate)
                    m1 = apool.tile([128, W], f16, tag="m1", name="m1")
                    nc.vector.tensor_tensor(m1[:], tg[:], sig[:, 0:W], OP.mult)
                    # sig(f), sig(o) off the critical path
                    nc.scalar.activation(
                        sig[:, W : 3 * W], st["gates"][:, W : 3 * W], AF.Sigmoid
                    )
                    if t > 0:
                        m2 = apool.tile([128, W], f16, tag="m2", name="m2")
                        nc.gpsimd.tensor_tensor(
                            m2[:], sig[:, W : 2 * W], st["c_prev"][:], OP.mult
                        )
                        st["m2"] = m2
                    st["sig"] = sig
                    st["m1"] = m1
                return fn

            def mk_s2(p, t):
                def fn():
                    st = state[p]
                    gates = st["gates"]
                    sig, m1 = st["sig"], st["m1"]
                    if t == 0:
                        c_new = m1  # c_0 = sig(i)*tanh(g), no add needed
                    else:
                        c_new = hcpool.tile([128, W], f16, tag="c", name="c")
                        nc.vector.tensor_tensor(
                            c_new[:], m1[:], st["m2"][:], OP.add
                        )
                    tanhc = apool.tile([128, W], f16, tag="tanhc", name="tanhc")
                    nc.scalar.activation(tanhc[:], c_new[:], AF.Tanh)
                    h_new = st["h_all"][:, ts(t, W)]
                    nc.vector.tensor_tensor(
                        h_new[:], sig[:, 2 * W : 3 * W], tanhc[:], OP.mult
                    )
                    if t < T - 1:
                        dh = apool.tile([128, W], f16, tag="dh", name="dh")
                        nc.vector.tensor_tensor(
                            dh[:], h_new[:], st["h_prev"][:], OP.subtract
                        )
                        for g in range(3):
                            nc.tensor.matmul(
                                gates[:, ts(g, W)], whht_s[:, ts(g, 128)], dh[:],
                                start=False, stop=(t == T - 2),
                                skip_group_check=True,
                            )
                            if t == 0:
                                nc.tensor.matmul(
                                    gates[:, ts(g, W)], wes_s[:, ts(g, 128)],
                                    st["k3rhs"][:],
                                    start=False, stop=False, skip_group_check=True,
                                )
                    st["h_prev"] = h_new
                    st["c_prev"] = c_new
                return fn

            def mk_rel(p):
                def fn():
                    st = state[p]
                    predsb = predpool.tile([128, 3 * W], f32, tag="predsb", name="predsb")
                    for q in range(3):
                        psr = scrpool.tile([128, W], f32, tag="scratch", name="psr")
                        for j in range(4):
                            nc.tensor.matmul(
                                psr[32 * j : 32 * j + 2, :], whpt_s[:],
                                st["h_all"][:, ts(4 * q + j, W)],
                                start=True, stop=True, skip_group_check=True,
                                tile_position=(0, 32 * j),
                            )
                        nc.vector.tensor_copy(
                            predsb[0:98, ts(q, W)], psr[0:98, :]
                        )
                    # predsb layout: [part 32j+{0,1}][q, col] ; t = 4q + j
                    src = predsb.rearrange("p (q b) -> p q b", q=3)
                    for j in range(4):
                        nc.sync.dma_start(
                            pred_v[:, j : j + 1, :, ts(p, W)].rearrange(
                                "q j c b -> c (j q) b"
                            ),
                            src[32 * j : 32 * j + 2, :, :],
                        )
                return fn

            # Staggered schedule: pair p's init rounds are s0, s0+1; steps at
            # s0+2+t; rel at s0+14.  MLP work is spread over the 9 preceding
            # rounds as PE filler.  Stagger 8 => 2 pair-chains in flight and
            # gate-tile reuse (bufs=2) has 1 round of slack.
            starts = [12 + 8 * p for p in range(NP)]
            for p in range(NP):
                s0 = starts[p]
                for j in range(8):
                    events.append((s0 - 11 + j, 40, mk_mlp1(p, j)))
                events.append((s0 - 3, 40, mk_mlp2(p)))
                events.append((s0 - 2, 39, mk_prep(p)))
                events.append((s0, 1, mk_init1(p)))
                events.append((s0 + 1, 1, mk_init2(p)))
                for t in range(T):
                    r = s0 + 2 + t
                    events.append((r, 2 + (T - t), mk_s1(p, t)))
                    events.append((r, 20 + (T - t), mk_s2(p, t)))
                events.append((s0 + 14, 2, mk_rel(p)))

            for _, _, fn in sorted(events, key=lambda e: (e[0], e[1])):
                fn()

    nc.compile()
    return nc


def _prep(inputs):
    """Host-side weight folding + layout prep. Returns per-core input maps."""
    f = np.float64
    W_ih = np.asarray(inputs["W_ih"], f)
    W_hh = np.asarray(inputs["W_hh"], f)
    b_ih = np.asarray(inputs["b_ih"], f)
    b_hh = np.asarray(inputs["b_hh"], f)
    W1 = np.asarray(inputs["W1"], f)
    b1 = np.asarray(inputs["b1"], f)
    W2 = np.asarray(inputs["W2"], f)
    b2 = np.asarray(inputs["b2"], f)
    W_sp = np.asarray(inputs["W_sp"], f)
    b_sp = np.asarray(inputs["b_sp"], f)
    W_hp = np.asarray(inputs["W_hp"], f)
    b_hp = np.asarray(inputs["b_hp"], f)

    W_zx = W_ih[:, :ZX]
    W_emb = W_ih[:, ZX:]
    W_es = W_emb @ W_sp                       # [4H, 2]
    W_hh_f = W_hh + W_es @ W_hp               # [4H, H]
    bias1 = b_ih + b_hh + W_emb @ b_sp + W_es @ b_hp

    # reorder pytorch gates (i, f, g, o) -> (i, f, o, g)
    perm = np.r_[0:H, H : 2 * H, 3 * H : 4 * H, 2 * H : 3 * H]
    W_zx = W_zx[perm]
    W_hh_f = W_hh_f[perm]
    W_es = W_es[perm]
    bias1 = bias1[perm]

    def kxm(Wt, kp):  # [K, M] -> [128, K/128, M] fp16, zero-padded to kp rows
        K, M = Wt.shape
        out = np.zeros((kp, M), f)
        out[:K] = Wt
        return np.ascontiguousarray(
            out.reshape(kp // 128, 128, M).transpose(1, 0, 2)
        ).astype(np.float16)

    # zx gate weights with bias1 folded at the constant-1.0 row (1056)
    W_zx_pad = np.zeros((KP, G4), f)
    W_zx_pad[:ZX] = W_zx.T
    W_zx_pad[ZX] = bias1

    consts = {
        "w1t": kxm(W1.T, KP),
        "wzxt": np.ascontiguousarray(
            W_zx_pad.reshape(KT, 128, G4).transpose(1, 0, 2)
        ).astype(np.float16),
        "w2t": kxm(W2.T, MLP),
        "whht": np.ascontiguousarray(W_hh_f.T).astype(np.float16),
        "whpt": np.ascontiguousarray(W_hp.T).astype(np.float16),
        "k3": np.ascontiguousarray(-W_es.T).astype(np.float16),
        "wes": np.ascontiguousarray(W_es[: 3 * H].T).astype(np.float16),
        "b1": np.ascontiguousarray(b1.reshape(8, 128).T).astype(np.float32),
        "b2": b2.reshape(128, 1).astype(np.float32),
        "bhp": b_hp.reshape(2, 1).astype(np.float32),
    }

    enc = np.asarray(inputs["enc_h_feat"], np.float32)
    z = np.asarray(inputs["z"], np.float32)
    lpr = np.asarray(inputs["last_pos_rel"], np.float32)
    zxT = np.zeros((KP, B), np.float16)
    zxT[:MLP] = enc.T
    zxT[MLP:ZX] = z.T
    zxT[ZX] = 1.0
    lprT = np.ascontiguousarray(lpr.T).astype(np.float16)

    in_maps = []
    for c in range(NCORES):
        s = slice(c * BC, (c + 1) * BC)
        m = dict(consts)
        m["zxT"] = np.ascontiguousarray(zxT[:, s])
        m["lprT"] = np.ascontiguousarray(lprT[:, s])
        in_maps.append(m)
    return in_maps


def run(inputs, trace=False):
    from concourse.bass_utils import run_bass_kernel_spmd

    if "nc" not in _cache:
        _cache["nc"] = _build_nc()
    in_maps = _prep(inputs)
    res = run_bass_kernel_spmd(
        _cache["nc"], in_maps, core_ids=list(range(NCORES)), trace=trace
    )
    pred = np.concatenate([r["pred"] for r in res.results], axis=2)  # [T, 2, B]
    out = pred.transpose(0, 2, 1) + np.asarray(inputs["b_hp"], np.float32)[None, None, :]
    return np.ascontiguousarray(out), res


def kernel(**inputs) -> np.ndarray:
    out, _ = run(inputs, trace=False)
    return out


# revision 22
# speedup vs baseline: 1.4008x; 1.4008x over previous
"""Trainium2 Bass kernel for the trajectory-decoder LSTM problem.

Math (mirrors the reference, with algebraic folds):
  dec_inp feeds gates only through W_emb; W_sp/W_emb/W_hp collapse:
    W_es = W_emb @ W_sp            [4H, 2]
    gates_t = zx@W_zx.T + bias + r_{t-1}@W_es.T + h_{t-1}@W_hh.T
  For t>=1, r_{t-1} = h_{t-1}@W_hp.T + b_hp, so with
    W_hh' = W_hh + W_es @ W_hp,  bias1 = b_ih + b_hh + W_emb@b_sp + W_es@b_hp
  every step becomes uniform:  gates_t = zx@W_zx.T + bias1 + h_{t-1}@W_hh'.T
  plus a rank-2 step-0 correction (lpr - r_init)@W_es.T injected once.
  `last_pos` is dead code (output is just the stacked rel_pos).

Device strategy (v2: 512-wide pair-waves, g-gate recomputed per step):
  - batch on the free dim, features on partitions; all matmuls N=512.
  - per 512-batch pair-wave, the i/f/o gate pre-activations live RESIDENT
    in a 3-bank PSUM tile; each step the PE accumulates
    (h_t - h_{t-1}) @ W_hh'[ifo].T into it.  The g (cell-candidate) gate
    is instead recomputed fresh each step into a transient 1-bank PSUM
    scratch slot: g_t = zxp_g(SBUF) + h_t @ W_hh'_g.T.  This shrinks the
    resident footprint so TWO pair-chains (2x3 banks) plus a 2-buf
    scratch pool (MLP/psr/g/rel) exactly fill the 8 PSUM banks.
  - bias1 is folded into the zx projection via a constant-1.0 row of the
    padded zx input (row 1056), killing the bank-open bias matmuls.
  - the g-gate weights are doubled on host so sigmoid(2g) gives
    tanh(g) = 2*sig(2g)-1 via the DVE ops; same trick for tanh(c) input.
  - rel_pos matmuls (M=2) are col-packed 4x with tile_position so four
    128x2x512 products run concurrently in different column groups.
  - waves are software-pipelined with a staggered round schedule
    (2 pair-chains in flight); Tile executes each engine's stream in
    emission order.
  - all matmul operands fp16 (full PE rate), PSUM accumulation fp32;
    c kept in fp16, b_hp added on host after gather.
"""

import os
import numpy as np

B = 32768
NCORES = 8
BC = B // NCORES          # 4096 batch per core
W = 512                   # batch per pair-chain
NP = BC // W              # 8 pair-waves per core
T = 12                    # decode steps
H = 128
G4 = 4 * H                # 512 gate features
G3 = 3 * H                # i,f,o resident gates
ZX = 1056
KP = 1152                 # ZX padded to 9*128 (row 1056 = 1.0 carries bias1)
KT = KP // 128            # 9 contraction tiles
MLP = 1024
EMB = 64

_cache = {}


def _build_nc():
    import concourse.bass as bass
    import concourse.bacc as bacc
    import concourse.mybir as mybir
    import concourse.tile as tile
    from concourse.bass import ts

    f16 = mybir.dt.float16
    f32 = mybir.dt.float32
    AF = mybir.ActivationFunctionType
    OP = mybir.AluOpType

    nc = bacc.Bacc("TRN2", target_bir_lowering=False)

    zxT = nc.dram_tensor("zxT", [KP, BC], f16, kind="ExternalInput")
    lprT = nc.dram_tensor("lprT", [2, BC], f16, kind="ExternalInput")
    w1t = nc.dram_tensor("w1t", [128, KT, MLP], f16, kind="ExternalInput")
    wzxt = nc.dram_tensor("wzxt", [128, KT, G4], f16, kind="ExternalInput")
    w2t = nc.dram_tensor("w2t", [128, 8, H], f16, kind="ExternalInput")
    whht = nc.dram_tensor("whht", [128, G4], f16, kind="ExternalInput")
    whpt = nc.dram_tensor("whpt", [128, 2], f16, kind="ExternalInput")
    k3 = nc.dram_tensor("k3", [2, G4], f16, kind="ExternalInput")   # -W_es.T
    wes = nc.dram_tensor("wes", [2, G3], f16, kind="ExternalInput")  # +W_es.T ifo
    ident = nc.dram_tensor("ident", [128, 128], f16, kind="ExternalInput")
    b1 = nc.dram_tensor("b1", [128, 8], f32, kind="ExternalInput")
    b2 = nc.dram_tensor("b2", [128, 1], f32, kind="ExternalInput")
    bhp = nc.dram_tensor("bhp", [2, 1], f32, kind="ExternalInput")
    pred = nc.dram_tensor("pred", [T, 2, BC], f32, kind="ExternalOutput")

    with tile.TileContext(nc) as tc:
        with (
            tc.tile_pool(name="consts", bufs=1) as cpool,
            tc.tile_pool(name="zx", bufs=2) as zxpool,
            tc.tile_pool(name="h1", bufs=2) as h1pool,
            tc.tile_pool(name="hc", bufs=5) as hcpool,
            tc.tile_pool(name="acts", bufs=4) as apool,
            tc.tile_pool(name="hall", bufs=2) as hallpool,
            tc.tile_pool(name="preds", bufs=2) as predpool,
            tc.tile_pool(name="scrps", bufs=2, space="PSUM") as scrpool,
            tc.tile_pool(name="gateps", bufs=2, space="PSUM") as gatepool,
        ):
            # ---- load constants once ----
            w1t_s = cpool.tile([128, KT, MLP], f16)
            nc.sync.dma_start(w1t_s[:], w1t[:])
            wzxt_s = cpool.tile([128, KT, G4], f16)
            nc.sync.dma_start(wzxt_s[:], wzxt[:])
            w2t_s = cpool.tile([128, 8, H], f16)
            nc.sync.dma_start(w2t_s[:], w2t[:])
            whht_s = cpool.tile([128, G4], f16)
            nc.sync.dma_start(whht_s[:], whht[:])
            whpt_s = cpool.tile([128, 2], f16)
            nc.sync.dma_start(whpt_s[:], whpt[:])
            k3_s = cpool.tile([2, G4], f16)
            nc.sync.dma_start(k3_s[:], k3[:])
            wes_s = cpool.tile([2, G3], f16)
            nc.sync.dma_start(wes_s[:], wes[:])
            ident_s = cpool.tile([128, 128], f16)
            nc.sync.dma_start(ident_s[:], ident[:])
            b1_s = cpool.tile([128, 8], f32)
            nc.sync.dma_start(b1_s[:], b1[:])
            b2_s = cpool.tile([128, 1], f32)
            nc.sync.dma_start(b2_s[:], b2[:])
            bhp_s = cpool.tile([2, 1], f32)
            nc.sync.dma_start(bhp_s[:], bhp[:])
            lpr_s = cpool.tile([2, BC], f16)
            nc.sync.dma_start(lpr_s[:], lprT[:])

            zxT_v = zxT.rearrange("(k p) b -> p k b", p=128)
            # pred viewed as [q, j, comp, b] with t = 4*q + j (rel col-packing)
            pred_v = pred.rearrange("(q j) c b -> q j c b", j=4)

            state = [dict() for _ in range(NP)]
            events = []  # (round, prio, fn)

            def mk_mlp1(p, j):
                def fn():
                    st = state[p]
                    if "zxw" not in st:
                        zxw = zxpool.tile([128, KT, W], f16, tag="zxw", name="zxw")
                        nc.sync.dma_start(zxw[:], zxT_v[:, :, ts(p, W)])
                        st["zxw"] = zxw
                        st["h1"] = h1pool.tile([128, 8, W], f16, tag="h1", name="h1")
                    ps = scrpool.tile([128, W], f32, tag="scratch", name="ps")
                    for k in range(KT):
                        nc.tensor.matmul(
                            ps[:], w1t_s[:, k, ts(j, 128)], st["zxw"][:, k, :],
                            start=(k == 0), stop=(k == KT - 1),
                        )
                    st["mlp_ps"] = ps
                return fn

            def mk_mlp1_relu(p, j):
                def fn():
                    st = state[p]
                    nc.vector.tensor_scalar(
                        st["h1"][:, j, :], st["mlp_ps"][:], b1_s[:, j : j + 1],
                        0.0, OP.add, OP.max,
                    )
                return fn

            def mk_mlp2(p):
                def fn():
                    st = state[p]
                    ps = scrpool.tile([128, W], f32, tag="scratch", name="ps")
                    for j in range(8):
                        nc.tensor.matmul(
                            ps[:], w2t_s[:, j, :], st["h1"][:, j, :],
                            start=(j == 0), stop=(j == 7),
                        )
                    st["mlp_ps"] = ps
                return fn

            def mk_mlp2_relu(p):
                def fn():
                    st = state[p]
                    hi = h1pool.tile([128, W], f16, tag="hinit", name="hinit")
                    nc.vector.tensor_scalar(
                        hi[:], st["mlp_ps"][:], b2_s[:, 0:1], 0.0, OP.add, OP.max
                    )
                    st["h_prev"] = hi
                return fn

            def mk_prep(p):
                # psr -> k3rhs, and zxp_g -> SBUF (g-gate zx projection)
                def fn():
                    st = state[p]
                    ps = scrpool.tile([128, W], f32, tag="scratch", name="ps")
                    nc.tensor.matmul(
                        ps[0:2, :], whpt_s[:], st["h_prev"][:], start=True, stop=True
                    )
                    k3rhs = apool.tile([2, W], f16, tag="k3rhs", name="k3rhs")
                    nc.vector.scalar_tensor_tensor(
                        k3rhs[:], ps[0:2, :], bhp_s[:, 0:1], lpr_s[:, ts(p, W)],
                        OP.add, OP.subtract,
                    )
                    st["k3rhs"] = k3rhs
                    ps2 = scrpool.tile([128, W], f32, tag="scratch", name="ps")
                    for k in range(KT):
                        nc.tensor.matmul(
                            ps2[:], wzxt_s[:, k, G3:G4], st["zxw"][:, k, :],
                            start=(k == 0), stop=(k == KT - 1),
                        )
                    zxpg = h1pool.tile([128, W], f16, tag="zxpg", name="zxpg")
                    nc.vector.tensor_copy(zxpg[:], ps2[:])
                    st["zxpg"] = zxpg
                return fn

            def mk_init(p):
                def fn():
                    st = state[p]
                    gates = gatepool.tile([128, G3 // 128 * W], f32, tag="gates", name="gates")
                    st["gates"] = gates
                    for g in range(3):
                        gp = gates[:, ts(g, W)]
                        for k in range(KT):
                            nc.tensor.matmul(
                                gp[:], wzxt_s[:, k, ts(g, 128)],
                                st["zxw"][:, k, :],
                                start=(k == 0), stop=False, skip_group_check=True,
                            )
                        nc.tensor.matmul(
                            gp[:], whht_s[:, ts(g, 128)], st["h_prev"][:],
                            start=False, stop=False, skip_group_check=True,
                        )
                        nc.tensor.matmul(
                            gp[:], k3_s[:, ts(g, 128)], st["k3rhs"][:],
                            start=False, stop=False, skip_group_check=True,
                        )
                    st["h_all"] = hallpool.tile([128, T * W], f16, tag="hall", name="hall")
                return fn

            def mk_s1(p, t):
                def fn():
                    st = state[p]
                    # fresh g-gate: psum <- h_prev @ Whh'_g (+ step-0 corr)
                    psg = scrpool.tile([128, W], f32, tag="scratch", name="psg")
                    nc.tensor.matmul(
                        psg[:], whht_s[:, G3:G4], st["h_prev"][:],
                        start=True, stop=False, skip_group_check=True,
                    )
                    # inject zxp_g via identity matmul -> tanh reads PSUM directly
                    nc.tensor.matmul(
                        psg[:], ident_s[:], st["zxpg"][:],
                        start=False, stop=(t > 0), skip_group_check=True,
                    )
                    if t == 0:
                        nc.tensor.matmul(
                            psg[:], k3_s[:, G3:G4], st["k3rhs"][:],
                            start=False, stop=True, skip_group_check=True,
                        )
                    sig = apool.tile([128, 3 * W], f16, tag="sig", name="sig")
                    # sig(i,f) first: they gate the critical path via m1/m2
                    nc.scalar.activation(
                        sig[:, 0 : 2 * W], st["gates"][:, 0 : 2 * W], AF.Sigmoid
                    )
                    tg = apool.tile([128, W], f16, tag="tg", name="tg")
                    nc.scalar.activation(tg[:], psg[:], AF.Tanh)
                    # m1 = sig(i) * tanh(g)   (plain tensor_tensor -> 2x DVE rate)
                    m1 = apool.tile([128, W], f16, tag="m1", name="m1")
                    nc.vector.tensor_tensor(m1[:], tg[:], sig[:, 0:W], OP.mult)
                    if t > 0:
                        m2 = apool.tile([128, W], f16, tag="m2", name="m2")
                        nc.gpsimd.tensor_tensor(
                            m2[:], sig[:, W : 2 * W], st["c_prev"][:], OP.mult
                        )
                        st["m2"] = m2
                    # sig(o) off the critical path
                    nc.scalar.activation(
                        sig[:, 2 * W : 3 * W], st["gates"][:, 2 * W : 3 * W], AF.Sigmoid
                    )
                    st["sig"] = sig
                    st["m1"] = m1
                return fn

            def mk_s2(p, t):
                def fn():
                    st = state[p]
                    gates = st["gates"]
                    sig, m1 = st["sig"], st["m1"]
                    if t == 0:
                        c_new = m1  # c_0 = sig(i)*tanh(g), no add needed
                    else:
                        c_new = hcpool.tile([128, W], f16, tag="c", name="c")
                        nc.vector.tensor_tensor(
                            c_new[:], m1[:], st["m2"][:], OP.add
                        )
                    tanhc = apool.tile([128, W], f16, tag="tanhc", name="tanhc")
                    nc.scalar.activation(tanhc[:], c_new[:], AF.Tanh)
                    h_new = st["h_all"][:, ts(t, W)]
                    nc.vector.tensor_tensor(
                        h_new[:], sig[:, 2 * W : 3 * W], tanhc[:], OP.mult
                    )
                    if t < T - 1:
                        dh = apool.tile([128, W], f16, tag="dh", name="dh")
                        nc.vector.tensor_tensor(
                            dh[:], h_new[:], st["h_prev"][:], OP.subtract
                        )
                        for g in range(3):
                            nc.tensor.matmul(
                                gates[:, ts(g, W)], whht_s[:, ts(g, 128)], dh[:],
                                start=False, stop=(t == T - 2),
                                skip_group_check=True,
                            )
                            if t == 0:
                                nc.tensor.matmul(
                                    gates[:, ts(g, W)], wes_s[:, ts(g, 128)],
                                    st["k3rhs"][:],
                                    start=False, stop=False, skip_group_check=True,
                                )
                    st["h_prev"] = h_new
                    st["c_prev"] = c_new
                return fn

            def mk_rel(p):
                def fn():
                    st = state[p]
                    predsb = predpool.tile([128, 3 * W], f32, tag="predsb", name="predsb")
                    for q in range(3):
                        psr = scrpool.tile([128, W], f32, tag="scratch", name="psr")
                        for j in range(4):
                            nc.tensor.matmul(
                                psr[32 * j : 32 * j + 2, :], whpt_s[:],
                                st["h_all"][:, ts(4 * q + j, W)],
                                start=True, stop=True, skip_group_check=True,
                                tile_position=(0, 32 * j),
                            )
                        nc.vector.tensor_copy(
                            predsb[0:98, ts(q, W)], psr[0:98, :]
                        )
                    # predsb layout: [part 32j+{0,1}][q, col] ; t = 4q + j
                    src = predsb.rearrange("p (q b) -> p q b", q=3)
                    for j in range(4):
                        nc.sync.dma_start(
                            pred_v[:, j : j + 1, :, ts(p, W)].rearrange(
                                "q j c b -> c (j q) b"
                            ),
                            src[32 * j : 32 * j + 2, :, :],
                        )
                return fn

            # Staggered schedule (per round, by prio): s1 of active chains
            # (3-14), init/mlp/prep PE blobs (16-18) fill the dh-wait gap in
            # the PE stream, s2 of active chains (21-32), rel (33), then the
            # deferred DVE relus (50).  Chain occupancy of a gate tile is 13
            # rounds (init, 12 steps w/ rel inside the last); stagger 7 keeps
            # two pair-chains almost fully overlapped with 1 round of slack
            # on tile reuse (bufs=2).
            starts = [13 + 7 * p for p in range(NP)]
            for p in range(NP):
                s0 = starts[p]
                for j in range(8):
                    events.append((s0 - 11 + j, 17, mk_mlp1(p, j)))
                    events.append((s0 - 11 + j, 50, mk_mlp1_relu(p, j)))
                events.append((s0 - 3, 17, mk_mlp2(p)))
                events.append((s0 - 3, 50, mk_mlp2_relu(p)))
                events.append((s0 - 2, 18, mk_prep(p)))
                events.append((s0, 16, mk_init(p)))
                for t in range(T):
                    r = s0 + 1 + t
                    events.append((r, 2 + (T - t), mk_s1(p, t)))
                    events.append((r, 20 + (T - t), mk_s2(p, t)))
                events.append((s0 + 12, 33, mk_rel(p)))

            for _, _, fn in sorted(events, key=lambda e: (e[0], e[1])):
                fn()

    nc.compile()
    return nc


def _prep(inputs):
    """Host-side weight folding + layout prep. Returns per-core input maps."""
    f = np.float64
    W_ih = np.asarray(inputs["W_ih"], f)
    W_hh = np.asarray(inputs["W_hh"], f)
    b_ih = np.asarray(inputs["b_ih"], f)
    b_hh = np.asarray(inputs["b_hh"], f)
    W1 = np.asarray(inputs["W1"], f)
    b1 = np.asarray(inputs["b1"], f)
    W2 = np.asarray(inputs["W2"], f)
    b2 = np.asarray(inputs["b2"], f)
    W_sp = np.asarray(inputs["W_sp"], f)
    b_sp = np.asarray(inputs["b_sp"], f)
    W_hp = np.asarray(inputs["W_hp"], f)
    b_hp = np.asarray(inputs["b_hp"], f)

    W_zx = W_ih[:, :ZX]
    W_emb = W_ih[:, ZX:]
    W_es = W_emb @ W_sp                       # [4H, 2]
    W_hh_f = W_hh + W_es @ W_hp               # [4H, H]
    bias1 = b_ih + b_hh + W_emb @ b_sp + W_es @ b_hp

    # reorder pytorch gates (i, f, g, o) -> (i, f, o, g)
    perm = np.r_[0:H, H : 2 * H, 3 * H : 4 * H, 2 * H : 3 * H]
    W_zx = W_zx[perm]
    W_hh_f = W_hh_f[perm]
    W_es = W_es[perm]
    bias1 = bias1[perm]

    def kxm(Wt, kp):  # [K, M] -> [128, K/128, M] fp16, zero-padded to kp rows
        K, M = Wt.shape
        out = np.zeros((kp, M), f)
        out[:K] = Wt
        return np.ascontiguousarray(
            out.reshape(kp // 128, 128, M).transpose(1, 0, 2)
        ).astype(np.float16)

    # zx gate weights with bias1 folded at the constant-1.0 row (1056)
    W_zx_pad = np.zeros((KP, G4), f)
    W_zx_pad[:ZX] = W_zx.T
    W_zx_pad[ZX] = bias1

    consts = {
        "w1t": kxm(W1.T, KP),
        "wzxt": np.ascontiguousarray(
            W_zx_pad.reshape(KT, 128, G4).transpose(1, 0, 2)
        ).astype(np.float16),
        "w2t": kxm(W2.T, MLP),
        "whht": np.ascontiguousarray(W_hh_f.T).astype(np.float16),
        "whpt": np.ascontiguousarray(W_hp.T).astype(np.float16),
        "k3": np.ascontiguousarray(-W_es.T).astype(np.float16),
        "wes": np.ascontiguousarray(W_es[: 3 * H].T).astype(np.float16),
        "ident": np.eye(128, dtype=np.float16),
        "b1": np.ascontiguousarray(b1.reshape(8, 128).T).astype(np.float32),
        "b2": b2.reshape(128, 1).astype(np.float32),
        "bhp": b_hp.reshape(2, 1).astype(np.float32),
    }

    enc = np.asarray(inputs["enc_h_feat"], np.float32)
    z = np.asarray(inputs["z"], np.float32)
    lpr = np.asarray(inputs["last_pos_rel"], np.float32)
    zxT = np.zeros((KP, B), np.float16)
    zxT[:MLP] = enc.T
    zxT[MLP:ZX] = z.T
    zxT[ZX] = 1.0
    lprT = np.ascontiguousarray(lpr.T).astype(np.float16)

    in_maps = []
    for c in range(NCORES):
        s = slice(c * BC, (c + 1) * BC)
        m = dict(consts)
        m["zxT"] = np.ascontiguousarray(zxT[:, s])
        m["lprT"] = np.ascontiguousarray(lprT[:, s])
        in_maps.append(m)
    return in_maps


def run(inputs, trace=False):
    from concourse.bass_utils import run_bass_kernel_spmd

    if "nc" not in _cache:
        _cache["nc"] = _build_nc()
    in_maps = _prep(inputs)
    res = run_bass_kernel_spmd(
        _cache["nc"], in_maps, core_ids=list(range(NCORES)), trace=trace
    )
    pred = np.concatenate([r["pred"] for r in res.results], axis=2)  # [T, 2, B]
    out = pred.transpose(0, 2, 1) + np.asarray(inputs["b_hp"], np.float32)[None, None, :]
    return np.ascontiguousarray(out), res


def kernel(**inputs) -> np.ndarray:
    out, _ = run(inputs, trace=False)
    return out
